# revision 50
# baseline (speedup 1.0000x reference)
"""Distributed (8-core) Trainium2 Bass kernel for nn_Attention.

Reference computation (per batch b of 4, x: [4, 256, 2048]):
  qkv = w_qkv @ x[b]            -> q,k,v each [8 heads, 64, 2048]
  dots = (q^T k) * 64**-0.5     -> [8, 2048, 2048]
  attn = softmax(dots, -1)
  av   = v @ attn^T             -> [8, 64, 2048]
  out  = w_out @ av + b_out     -> [256, 2048]

Sharding: 8 shards = (batch b in 0..3) x (query-half in 0..1). Each core
gets the full x[b] (columns permuted so its own 1024 query positions come
first), computes full k/v (duplicated with its half-partner), q only for
its 1024 queries, its half of the attention, and its half of the final
projection. Host concatenates.

Per-core softmax exp is the bottleneck (16.8M elements; ScalarE alone
would take ~147us). It is split between ScalarE (exact ACT exp,
~1147ns per [128,1024] tile) and DVE (3 of every 16 tiles; Schraudolph
bit-trick exp: one tensor_scalar computing int16(round(d*A + B)) whose
bits ARE bf16(exp(d*SCALE)) with a +-3% sawtooth error that averages
out across keys; ~1.2us/tile). The sawtooth mean is zeroed via the B
constant so mixing exact and approx tiles inside one softmax row does
not bias the attention weights. DVE-exp tiles' AV matmuls are emitted
one jc late so the in-order PE FIFO never waits on the DVE queue.

Softmax normalization: both heads' denominator rows are copied to a
partition-0 [1,1024] tile (the reciprocal custom uop breaks at base
partition != 0 on HW), one batched reciprocal_approx_fast, then the
reciprocal row is broadcast across 64 partitions by bouncing through
DRAM with a 0-stride-source DMA (gpsimd's partition_broadcast is
loadable ext-isa ucode that thrashes a ~6us IRAM reload against its
resident tensor_tensor; a 0-stride SBUF DMA source is illegal), and
the per-head multiply runs on gpsimd (resident ucode, exact). The last
group instead broadcasts via a PE fp32 outer product and multiplies on
DVE to keep the tail chain short.

Emission is paced: the v-projection, next pair's q/k projections,
normalization chain and output projection are drained one-per-jc inside
the attention jc loops so the PE/DVE/ScalarE pipelines never see a burst
at group boundaries. PSUM: 4 banks dots (double-buffered [128,1024]),
2 banks AV accumulators, 2 banks projections (so a projection matmul
never WAR-serializes on the previous piece's cast). Junk warm-up
matmuls run during the input DMA window so the PE HAM clock-gate is
released before the first real matmul burst.
"""

import sys

sys.path.insert(0, "/opt/trn_rl_repo")
sys.path.insert(0, "/root/.axon_site")

from collections import deque

import numpy as np

DIM = 256
N = 2048
NQ = 1024
H = 8
DH = 64
HID = 512
PAIRS = 4
SCALE = DH ** -0.5

# Schraudolph exp constants: int16 bits of bf16(exp(d*SCALE)) ~=
# round(d * A_S + B_S).  B centered so the relative-error sawtooth is
# mean-zero (c = -0.0564 -> 128*c = -7.2193).
A_S = 128.0 * np.log2(np.e) * SCALE
B_S = 127.0 * 128.0 - 7.2192745

# jc indices (of 16) whose exp runs on DVE instead of ScalarE
DVE_JC = (3, 8, 13)

_CACHE = {}


def _register_ntff_hook():
    """The agent image's antenv lacks axon_hooks; synthesize it so
    run_bass_kernel_spmd(trace=True) can profile. Harmless if unused."""
    import types

    if "antenv.axon_hooks" in sys.modules:
        return
    try:
        import antenv
        from trn_agent_boot.trn_boot import _ntff_profile_via_ctypes

        mod = types.ModuleType("antenv.axon_hooks")
        _hook = [None]
        mod.set_axon_ntff_profile_hook = lambda h: _hook.__setitem__(0, h)
        mod.get_axon_ntff_profile_hook = lambda: _hook[0]
        sys.modules["antenv.axon_hooks"] = mod
        antenv.axon_hooks = mod
        mod.set_axon_ntff_profile_hook(
            _ntff_profile_via_ctypes("/opt/axon/libaxon_pjrt.so")
        )
    except Exception:
        pass


def build_nc():
    import concourse.mybir as mybir
    import concourse.tile as tile
    from concourse import bacc

    f32 = mybir.dt.float32
    bf16 = mybir.dt.bfloat16
    i16 = mybir.dt.int16
    Exp = mybir.ActivationFunctionType.Exp
    MULT = mybir.AluOpType.mult
    ADD = mybir.AluOpType.add

    nc = bacc.Bacc("TRN2", target_bir_lowering=False, debug=False)

    x_ext = nc.dram_tensor("x", [DIM, N], bf16, kind="ExternalInput")
    # wq_t | wk_t | wv_t packed along columns: fewer, larger startup DMAs
    wqkv_ext = nc.dram_tensor("wqkv_t", [DIM, 3 * HID], bf16, kind="ExternalInput")
    wo_ext = nc.dram_tensor("wo_t", [HID, DIM], bf16, kind="ExternalInput")
    b_ext = nc.dram_tensor("bias", [DIM, 1], f32, kind="ExternalInput")
    out_ext = nc.dram_tensor("out", [DIM, NQ], bf16, kind="ExternalOutput")
    # DRAM bounce buffer for the softmax-denominator reciprocal broadcast:
    # row g holds [recA | recB] for group g
    rb_ext = nc.dram_tensor("rbounce", [8, 1024], f32, kind="Internal")

    # v slot layout: [ones | 63 dead cols | 64 v cols]. The AV matmul then
    # writes [128, 512] with the softmax denominator on PSUM partition 0
    # (the reciprocal custom uop only works at base partition 0, so it can
    # read the den row directly -- no staging copies) and the values at
    # partitions 64:128 (a legal quadrant base for the normalize multiply).
    # Dead cols are never read downstream; m-width is free on the PE
    # (cycles = rhs columns).
    VSLOT = 2 * DH

    with tile.TileContext(nc) as tc:
        with (
            tc.tile_pool(name="persist", bufs=1) as pp,
            tc.tile_pool(name="qk", bufs=2) as qk,
            tc.tile_pool(name="epool", bufs=5) as ep,
            tc.tile_pool(name="eipool", bufs=3) as eip,
            tc.tile_pool(name="small", bufs=4) as sp,
            tc.tile_pool(name="pdots", bufs=2, space="PSUM") as pd,
            tc.tile_pool(name="pattn", bufs=2, space="PSUM") as pa,
            # bufs=2: a projection matmul must not serialize behind the
            # previous piece's PSUM->SBUF cast (WAR on a single bank)
            tc.tile_pool(name="pproj", bufs=2, space="PSUM") as pj,
        ):
            # ---- input DMAs (bf16, host pre-transposed/cast) ----
            # NOTE: emitted before the ACT-table warm-up — the scalar ring
            # carries x, and its descriptors must not queue behind the
            # ~2.7us ACT_TABLE_LOAD
            def load_bf16(ext, rows, cols, tag):
                tiles = []
                h = cols // 2
                for cc in range(rows // 128):
                    t = pp.tile([128, cols], bf16, tag=f"{tag}{cc}", name=f"{tag}{cc}")
                    r0, r1 = cc * 128, (cc + 1) * 128
                    nc.sync.dma_start(t[:, 0:h], ext[r0:r1, 0:h])
                    nc.gpsimd.dma_start(t[:, h:cols], ext[r0:r1, h:cols])
                    tiles.append(t)
                return tiles

            # Priority load order. Group (0,0) starts once wqkv q0/k0 cols
            # + x[*, 0:512] land, and consumes all v-columns + x
            # progressively (jit v + k pieces), so: scalar ring carries
            # x[*, 0:512] first (256KB, the critical path), sync/gpsimd
            # carry each cc's pair-0-critical wqkv columns
            # [q0 0:128 | k0 512:640 | v 1024:1536], then the remaining x
            # halves, then the rest of wqkv, wo and bias.
            wqkv_sb = [
                pp.tile([128, 3 * HID], bf16, tag=f"wqkv{c}", name=f"wqkv{c}")
                for c in range(2)
            ]
            x_sb = [pp.tile([128, N], bf16, tag=f"x{c}", name=f"x{c}") for c in range(2)]
            # Priority load order. Group (0,0) starts once wqkv q0/k0 cols
            # + x[*, 0:512] land, and consumes all v-columns + x
            # progressively (jit v + k pieces), so: scalar ring carries
            # x[*, 0:512] first (256KB, the critical path), sync/gpsimd
            # carry each cc's pair-0-critical wqkv columns
            # [q0 0:128 | k0 512:640 | v 1024:1536], then the remaining x
            # halves, then the rest of wqkv, wo and bias.
            engs = [nc.sync, nc.gpsimd]
            for c in range(2):
                r0, r1 = c * 128, (c + 1) * 128
                nc.scalar.dma_start(x_sb[c][:, 0:512], x_ext[r0:r1, 0:512])
                engs[c].dma_start(
                    wqkv_sb[c][:, 0:128], wqkv_ext[r0:r1, 0:128]
                )
                engs[c].dma_start(
                    wqkv_sb[c][:, HID : HID + 128],
                    wqkv_ext[r0:r1, HID : HID + 128],
                )
                engs[c].dma_start(
                    wqkv_sb[c][:, 2 * HID : 3 * HID],
                    wqkv_ext[r0:r1, 2 * HID : 3 * HID],
                )
                engs[c].dma_start(
                    x_sb[c][:, 512:1024], x_ext[r0:r1, 512:1024]
                )
            nc.scalar.dma_start(x_sb[0][:, 1024:2048], x_ext[0:128, 1024:2048])
            nc.gpsimd.dma_start(x_sb[1][:, 1024:1536], x_ext[128:256, 1024:1536])
            nc.scalar.dma_start(x_sb[1][:, 1536:2048], x_ext[128:256, 1536:2048])
            for c in range(2):
                r0, r1 = c * 128, (c + 1) * 128
                engs[c].dma_start(
                    wqkv_sb[c][:, 128:HID], wqkv_ext[r0:r1, 128:HID]
                )
                engs[c].dma_start(
                    wqkv_sb[c][:, HID + 128 : 2 * HID],
                    wqkv_ext[r0:r1, HID + 128 : 2 * HID],
                )
            wo_sb = load_bf16(wo_ext, HID, DIM, "wo")
            bias_sb = pp.tile([128, 2], f32, tag="bias")
            for oc in range(2):
                nc.sync.dma_start(
                    bias_sb[:, oc : oc + 1], b_ext[oc * 128 : (oc + 1) * 128, :]
                )
            # ---- warm the ACT exp table early (one tiny op) ----
            dummy = sp.tile([1, 1], f32, tag="dummy")
            nc.vector.memset(dummy[:], 0.0)
            dummy2 = sp.tile([1, 1], f32, tag="dummy2")
            nc.scalar.activation(dummy2[:], dummy[:], Exp)

            # two junk matmuls fill the PE's pre-DMA dead window (~7.4-8.7us)
            # and start the HAM warm-up; the real q/k/v projections (ready
            # ~8.8us) continue it -- 8 junk matmuls measured 3.5us of
            # head-of-line delay on the first real projection
            wu = pp.tile([1, 512], bf16, tag="wu")
            nc.vector.memset(wu[:], 1.0)
            for _ in range(5):
                wps = pj.tile([64, 512], f32, tag="proj", name="wps")
                nc.tensor.matmul(
                    wps[:], lhsT=wu[:, 0:64], rhs=wu[:], start=True, stop=True
                )

            # ---- persistent tiles ----
            vt = pp.tile([128, 16 * H * VSLOT], bf16, tag="vt")
            # ones column FIRST in each slot so the AV accumulator's
            # denominator row lands on PSUM partition 0 (reciprocal custom
            # uop requires base partition 0; lets it read PSUM directly)
            vt_slots = vt[:].rearrange("p (s v) -> p s v", v=VSLOT)
            nc.vector.memset(vt_slots[:, :, 0:1], 1.0)

            attn_n = [
                pp.tile([128, NQ], bf16, tag=f"attn_n{p}", name=f"attn_n{p}")
                for p in range(PAIRS)
            ]
            ones64b = pp.tile([1, DH], bf16, tag="ones64b")
            nc.vector.memset(ones64b[:], 1.0)
            out_acc = [
                pp.tile([128, NQ], f32, tag=f"oacc{oc}", name=f"oacc{oc}")
                for oc in range(2)
            ]

            # ---- q/k projection pieces (matmul and cast as separate paced
            # steps: the cast runs a jc later so it never makes the DVE
            # queue wait on a just-issued matmul) ----
            def qk_mm(p, q_t, k_t, which, pool, tag, box):
                kind, col = which
                off = 0 if kind == "q" else HID
                ps = pool.tile([128, 512], f32, tag=tag, name="ps")
                for cc in range(2):
                    nc.tensor.matmul(
                        ps[:],
                        lhsT=wqkv_sb[cc][:, off + p * 128 : off + (p + 1) * 128],
                        rhs=x_sb[cc][:, col : col + 512],
                        start=(cc == 0),
                        stop=(cc == 1),
                    )
                box[which] = ps

            def qk_cast(q_t, k_t, which, box):
                kind, col = which
                dst = q_t if kind == "q" else k_t
                nc.vector.tensor_copy(dst[:, col : col + 512], box.pop(which)[:])

            def qk_piece(p, q_t, k_t, which, pool, tag):
                box = {}
                qk_mm(p, q_t, k_t, which, pool, tag, box)
                qk_cast(q_t, k_t, which, box)

            def qk_steps(p, q_t, k_t, whichs, pool, tag):
                """One paced closure per projection chunk. (A split
                mm/cast interleave was tried and measured worse: the extra
                paced items pushed the DVE exp tiles further back in its
                FIFO.)"""
                return [
                    (lambda which=which: qk_piece(p, q_t, k_t, which, pool, tag))
                    for which in whichs
                ]

            def v_piece(jc):
                ps = pj.tile([128, HID], f32, tag="proj", name="ps")
                for cc in range(2):
                    nc.tensor.matmul(
                        ps[:],
                        lhsT=x_sb[cc][:, jc * 128 : (jc + 1) * 128],
                        rhs=wqkv_sb[cc][:, 2 * HID : 3 * HID],
                        start=(cc == 0),
                        stop=(cc == 1),
                    )
                vslice = vt[
                    :, jc * H * VSLOT : (jc + 1) * H * VSLOT
                ].rearrange("p (h s) -> p h s", s=VSLOT)
                nc.vector.tensor_copy(
                    vslice[:, :, DH : 2 * DH],
                    ps[:].rearrange("p (h d) -> p h d", d=DH),
                )

            # ---- pair 0 q/k: ic0-critical pieces upfront (pd pool), rest
            # paced into the first attention group (pj pool) ----
            q0_t = qk.tile([128, NQ], bf16, tag="q", name="q_t")
            k0_t = qk.tile([128, N], bf16, tag="k", name="k_t")
            qk_piece(0, q0_t, k0_t, ("q", 0), pd, "dots")
            qk_piece(0, q0_t, k0_t, ("k", 0), pd, "dots")

            def norm_chain(p, ic, att_s_pair, att_pair):
                """Normalization closures for one (p, ic). The AV ones row
                sits at PSUM/att_s partition 0, so the reciprocal custom
                uop (base-partition-0 only) runs straight on the den row:
                no den staging copies. rec is broadcast across 64
                partitions by bouncing through DRAM with a 0-stride-source
                DMA (gpsimd's partition_broadcast is ext-isa ucode that
                would thrash its resident tensor_tensor; a 0-stride SBUF
                DMA source is illegal), and the per-head multiply runs on
                gpsimd (resident ucode, exact). The last group instead
                reads everything straight from PSUM (recip + DVE muls) and
                broadcasts via a PE outer product to keep the tail short."""
                last = p == PAIRS - 1 and ic == 1
                g = 2 * p + ic
                box = {}

                den_src = att_pair if last else att_s_pair

                def make_rec(hh):
                    def s():
                        if hh == 0:
                            rec = sp.tile([1, 1024], f32, tag="rec", name="rec")
                            box["rec"] = rec
                        rec = box["rec"]
                        nc.vector.reciprocal_approx_fast(
                            rec[:, hh * 512 : (hh + 1) * 512],
                            den_src[hh][0:1, :],
                        )
                        if hh == 1 and not last:
                            nc.sync.dma_start(rb_ext[g : g + 1, :], rec[:])
                    return s

                def s_cast():
                    # last group only: bf16 rec so the PE broadcast runs
                    # at bf16 column rate (one [64,1024] matmul for both
                    # heads vs 2x 1.06us fp32)
                    rec_b = sp.tile([1, 1024], bf16, tag="rec_b")
                    nc.vector.tensor_copy(rec_b[:], box["rec"][:])
                    box["rec_b"] = rec_b

                def s_bcast_pe():
                    # two matmuls: a single one may not write across a
                    # PSUM bank boundary (1024 fp32 cols = 2 banks)
                    recb = pd.tile([128, 1024], f32, tag="dots", name="recb_ps")
                    for hh in range(2):
                        nc.tensor.matmul(
                            recb[0:64, hh * 512 : (hh + 1) * 512],
                            lhsT=ones64b[:],
                            rhs=box["rec_b"][:, hh * 512 : (hh + 1) * 512],
                            start=True,
                            stop=True,
                        )
                    box["recb"] = recb

                def make_bcast(hh):
                    def s():
                        # written at partitions 64:128: TensorTensor
                        # requires equal base partitions when both
                        # inputs are SBUF, and the att values sit at
                        # 64:128
                        recb = sp.tile([128, 512], f32, tag="recb")
                        nc.sync.dma_start(
                            recb[64:128, :],
                            rb_ext[
                                g : g + 1, hh * 512 : (hh + 1) * 512
                            ].broadcast_to([64, 512]),
                        )
                        box[f"recb{hh}"] = recb
                    return s

                def make_mul(hh):
                    def s():
                        if last:
                            # DVE with the PE-broadcast recb still in PSUM
                            # (one PSUM operand is legal, two are not)
                            nc.vector.tensor_tensor(
                                attn_n[p][
                                    hh * 64 : (hh + 1) * 64,
                                    ic * 512 : (ic + 1) * 512,
                                ],
                                att_s_pair[hh][DH : 2 * DH, :],
                                box["recb"][0:64, hh * 512 : (hh + 1) * 512],
                                mybir.AluOpType.mult,
                            )
                        else:
                            recb = box[f"recb{hh}"]
                            nc.gpsimd.tensor_tensor(
                                attn_n[p][
                                    hh * 64 : (hh + 1) * 64,
                                    ic * 512 : (ic + 1) * 512,
                                ],
                                att_s_pair[hh][DH : 2 * DH, :],
                                recb[64:128, :],
                                mybir.AluOpType.mult,
                            )
                    return s

                if last:
                    return [
                        make_rec(0), make_rec(1), s_cast, s_bcast_pe,
                        make_mul(0), make_mul(1),
                    ]
                return [
                    make_rec(0), make_rec(1),
                    make_bcast(0), make_mul(0),
                    make_bcast(1), make_mul(1),
                ]

            def outproj_steps(p, ic2):
                """Output projection for pair p, query-half ic2. The ic
                attention groups produce attn_n column halves, so the ic2=0
                projection can pace a full group earlier than ic2=1."""
                steps = []
                for oc in range(2):
                    def s(oc=oc):
                        ps = pj.tile([128, 512], f32, tag="proj", name="ops")
                        nc.tensor.matmul(
                            ps[:],
                            lhsT=wo_sb[p][:, oc * 128 : (oc + 1) * 128],
                            rhs=attn_n[p][:, ic2 * 512 : (ic2 + 1) * 512],
                            start=True,
                            stop=True,
                        )
                        dst = out_acc[oc][:, ic2 * 512 : (ic2 + 1) * 512]
                        if p == 0:
                            nc.vector.tensor_scalar_add(
                                dst, ps[:], bias_sb[:, oc : oc + 1]
                            )
                        elif p < PAIRS - 1:
                            nc.vector.tensor_add(dst, dst, ps[:])
                        else:
                            # last pair: final sum goes to a bf16 staging
                            # tile (halves the output DMA bytes) and ships
                            # split across rings. Scalar's ring only joins
                            # for the drain-time half (it is exp-busy
                            # while ic2=0 paces through group (3,1)).
                            stage = sp.tile(
                                [128, 512], bf16, tag="ostage", name="ostage"
                            )
                            nc.vector.tensor_add(stage[:], dst, ps[:])
                            orow = out_ext[oc * 128 : (oc + 1) * 128, :]
                            c0 = ic2 * 512
                            if ic2 == 0:
                                nc.sync.dma_start(
                                    orow[:, c0 : c0 + 256], stage[:, 0:256]
                                )
                                nc.gpsimd.dma_start(
                                    orow[:, c0 + 256 : c0 + 512],
                                    stage[:, 256:512],
                                )
                            else:
                                nc.sync.dma_start(
                                    orow[:, c0 : c0 + 192], stage[:, 0:192]
                                )
                                nc.gpsimd.dma_start(
                                    orow[:, c0 + 192 : c0 + 384],
                                    stage[:, 192:384],
                                )
                                nc.scalar.dma_start(
                                    orow[:, c0 + 384 : c0 + 512],
                                    stage[:, 384:512],
                                )
                    steps.append(s)
                return steps

            # ---- attention groups ----
            pending = deque()
            # remaining pair-0 projection pieces, paced into group (0, 0).
            # k first: group (0,0)'s own dots consume k chunk jc*128 at
            # iteration jc, so each k cast must drain before its first
            # reader (k512 cast lands at drain slot 3 < jc4, k1024 at 6 <
            # jc8, k1536 at 8 < jc12).
            pending.extend(
                qk_steps(
                    0, q0_t, k0_t,
                    (("k", 512), ("q", 512), ("k", 1024), ("k", 1536)),
                    pj, "proj",
                )
            )

            cur_qk = (q0_t, k0_t)
            next_qk = [None]

            def attention_group(p, ic, jit_v, pre_d=None, next_prologue=None):
                """Software-pipelined: dots(jc+1) is emitted BEFORE AV(jc)
                and every exp is emitted before any same-jc AV. Tile's
                cross-engine waits use a conservative PE-completion counter
                (everything emitted so far), so an exp emitted after an AV
                transitively waits for that AV (+ ~0.6us semaphore latency)
                -- that chain, not ScalarE throughput, set the old 1.19us
                steady period. With dots hoisted ahead, exp(jc+1)'s
                threshold covers only work that finishes during exp(jc).
                next_prologue (the next group's dots(0)) is emitted at
                jc=15 so the boundary AV(15) doesn't head-of-line block
                the next group's start."""
                # group (0,0) already loads DVE with the 16 v-copies; keep
                # its exps on ScalarE there
                dve_jc = () if jit_v else DVE_JC
                q_t, k_t = cur_qk
                attA = pa.tile([128, 512], f32, tag="att", name="attA")
                attB = pa.tile([128, 512], f32, tag="att", name="attB")

                def av_pair(jc, eA, eB):
                    base = jc * H * VSLOT
                    hA = 2 * p
                    hB = 2 * p + 1
                    nc.tensor.matmul(
                        attA[:],
                        lhsT=vt[:, base + hA * VSLOT : base + (hA + 1) * VSLOT],
                        rhs=eA,
                        start=(jc == 0),
                        stop=(jc == 15),
                    )
                    nc.tensor.matmul(
                        attB[:],
                        lhsT=vt[:, base + hB * VSLOT : base + (hB + 1) * VSLOT],
                        rhs=eB,
                        start=(jc == 0),
                        stop=(jc == 15),
                    )

                d_tiles = {}

                def emit_dots(jc):
                    if jit_v:
                        v_piece(jc)
                    d = pd.tile([128, 1024], f32, tag="dots", name="d")
                    nc.tensor.matmul(
                        d[:, 0:512],
                        lhsT=k_t[0:64, jc * 128 : (jc + 1) * 128],
                        rhs=q_t[0:64, ic * 512 : (ic + 1) * 512],
                        start=True,
                        stop=True,
                    )
                    nc.tensor.matmul(
                        d[:, 512:1024],
                        lhsT=k_t[64:128, jc * 128 : (jc + 1) * 128],
                        rhs=q_t[64:128, ic * 512 : (ic + 1) * 512],
                        start=True,
                        stop=True,
                    )
                    d_tiles[jc] = d

                if pre_d is not None:
                    d_tiles[0] = pre_d.pop("d")
                else:
                    emit_dots(0)
                deferred_av = []
                for jc in range(16):
                    d = d_tiles.pop(jc)
                    # exp first: its PE-counter threshold must not cover
                    # the AVs emitted below
                    if jc in dve_jc:
                        ei = eip.tile([128, 1024], i16, tag="ei")
                        nc.vector.tensor_scalar(
                            ei[:], d[:], A_S, B_S, MULT, ADD
                        )
                        eA = ei[:, 0:512].bitcast(bf16)
                        eB = ei[:, 512:1024].bitcast(bf16)
                        this_av = (
                            lambda jc=jc, eA=eA, eB=eB: av_pair(jc, eA, eB)
                        )
                    else:
                        e = ep.tile([128, 1024], bf16, tag="e")
                        nc.scalar.activation(e[:], d[:], Exp, scale=SCALE)
                        this_av = (
                            lambda jc=jc, e=e: av_pair(
                                jc, e[:, 0:512], e[:, 512:1024]
                            )
                        )
                    # next dots ahead of any AV in the PE FIFO
                    if jc < 15:
                        emit_dots(jc + 1)
                    elif next_prologue is not None:
                        next_prologue()
                    # a deferred AV slots in here: its exp (on DVE) got
                    # nearly two jcs of latency without stalling PE's FIFO
                    for av in deferred_av:
                        av()
                    deferred_av.clear()
                    if jc in dve_jc and jc < 15:
                        deferred_av.append(this_av)
                    else:
                        this_av()
                    # pace deferred side-steps: one per jc, plus extras if
                    # the backlog wouldn't drain by the end of the group
                    if jc >= 1:
                        popped = 0
                        while pending and (
                            popped < 1 or len(pending) > 15 - jc
                        ):
                            pending.popleft()()
                            popped += 1
                for av in deferred_av:
                    av()
                # free the accumulator banks quickly with one copy per head;
                # on DVE — moving these to ScalarE measured 3-5us worse (the
                # copy head-of-line blocks ScalarE's FIFO at group
                # boundaries while it waits for the AV stop).
                # last group: copies on ScalarE (idle by then) so the DVE
                # FIFO goes straight to the recip chain
                last = p == PAIRS - 1 and ic == 1
                att_s_pair = []
                for att in (attA, attB):
                    att_s = sp.tile([128, 512], f32, tag="att_s", name="att_s")
                    if last:
                        nc.scalar.copy(att_s[:], att[:])
                    else:
                        nc.vector.tensor_copy(att_s[:], att[:])
                    att_s_pair.append(att_s)
                return att_s_pair, (attA, attB)

            def make_prologue(p_n, ic_n, qk_n):
                """Next group's dots(0), emitted at the current group's
                jc=15 so it runs in the last exp's shadow instead of
                queueing behind the boundary AV(15)."""
                box = {}

                def prologue():
                    q_t, k_t = qk_n
                    d = pd.tile([128, 1024], f32, tag="dots", name="d")
                    nc.tensor.matmul(
                        d[:, 0:512],
                        lhsT=k_t[0:64, 0:128],
                        rhs=q_t[0:64, ic_n * 512 : (ic_n + 1) * 512],
                        start=True,
                        stop=True,
                    )
                    nc.tensor.matmul(
                        d[:, 512:1024],
                        lhsT=k_t[64:128, 0:128],
                        rhs=q_t[64:128, ic_n * 512 : (ic_n + 1) * 512],
                        start=True,
                        stop=True,
                    )
                    box["d"] = d

                return prologue, box

            groups = [(p, ic) for p in range(PAIRS) for ic in range(2)]
            pre_d = None
            for gi, (p, ic) in enumerate(groups):
                if ic == 0 and p < PAIRS - 1:
                    # queue next pair's q/k projection pieces: first 3
                    # paced into this (p, ic0) group, rest into (p, ic1).
                    # Group (0,0) is already loaded with the jit
                    # v-projection, so pair 1's pieces all go to (0,1).
                    nq_t = qk.tile([128, NQ], bf16, tag="q", name="q_t")
                    nk_t = qk.tile([128, N], bf16, tag="k", name="k_t")
                    next_qk[0] = (nq_t, nk_t)
                    first = () if p == 0 else (("q", 0), ("k", 0), ("q", 512))
                    if first:
                        pending.extend(
                            qk_steps(p + 1, nq_t, nk_t, first, pj, "proj")
                        )
                if ic == 1 and p < PAIRS - 1:
                    nq_t, nk_t = next_qk[0]
                    rest = (
                        (("q", 0), ("k", 0), ("q", 512),
                         ("k", 512), ("k", 1024), ("k", 1536))
                        if p == 0
                        else (("k", 512), ("k", 1024), ("k", 1536))
                    )
                    pending.extend(
                        qk_steps(p + 1, nq_t, nk_t, rest, pj, "proj")
                    )
                if gi + 1 < len(groups):
                    p_n, ic_n = groups[gi + 1]
                    qk_n = cur_qk if ic_n == 1 else next_qk[0]
                    next_prologue, next_box = make_prologue(p_n, ic_n, qk_n)
                else:
                    next_prologue, next_box = None, None
                att_s_pair, att_pair = attention_group(
                    p, ic, jit_v=(p == 0 and ic == 0),
                    pre_d=pre_d, next_prologue=next_prologue,
                )
                pre_d = next_box
                for s in norm_chain(p, ic, att_s_pair, att_pair):
                    pending.append(s)
                if p == PAIRS - 1 and ic == 0:
                    # last pair: the first query-half's output projection
                    # paces into group (3,1) so the final drain only holds
                    # norm(3,1) + outproj(3,1)
                    for s in outproj_steps(p, 0):
                        pending.append(s)
                if ic == 1:
                    # both query halves' projections pace into the next
                    # pair's first group (pacing them earlier measured
                    # worse: they crowd the qk-piece groups)
                    if p == PAIRS - 1:
                        for s in outproj_steps(p, 1):
                            pending.append(s)
                    else:
                        for s in outproj_steps(p, 0) + outproj_steps(p, 1):
                            pending.append(s)
                    if p < PAIRS - 1:
                        cur_qk = next_qk[0]
            # drain whatever is left (last pair's norm + output projection)
            while pending:
                pending.popleft()()

    nc.compile()
    return nc


def _shard_inputs(x, w_qkv, w_out, b_out):
    """Returns in_maps for cores 0..7; core c = (batch c//2, query-half c%2)."""
    x = np.asarray(x, dtype=np.float32)
    w_qkv = np.asarray(w_qkv, dtype=np.float32)
    w_out = np.asarray(w_out, dtype=np.float32)
    b_out = np.asarray(b_out, dtype=np.float32)

    import ml_dtypes

    bf = ml_dtypes.bfloat16
    # wq_t | wk_t | wv_t packed: [256, 1536]
    wqkv_t = np.ascontiguousarray(w_qkv.T).astype(bf)
    wo_t = np.ascontiguousarray(w_out.T).astype(bf)  # [512, 256]
    bias = np.ascontiguousarray(b_out.reshape(DIM, 1))

    in_maps = []
    for c in range(8):
        b, half = divmod(c, 2)
        xb = x[b]
        halves = [xb[:, 0:NQ], xb[:, NQ:N]]
        x_perm = np.ascontiguousarray(
            np.concatenate([halves[half], halves[1 - half]], axis=1)
        ).astype(ml_dtypes.bfloat16)
        in_maps.append(
            {
                "x": x_perm,
                "wqkv_t": wqkv_t,
                "wo_t": wo_t,
                "bias": bias,
            }
        )
    return in_maps


def run(x, w_qkv, w_out, b_out, trace=False, tmpdir=None):
    from concourse.bass_utils import run_bass_kernel_spmd

    _register_ntff_hook()
    if "nc" not in _CACHE:
        _CACHE["nc"] = build_nc()
    nc = _CACHE["nc"]
    in_maps = _shard_inputs(x, w_qkv, w_out, b_out)
    kw = {}
    if trace:
        kw.update(trace=True, tmpdir=tmpdir)
    res = run_bass_kernel_spmd(nc, in_maps, core_ids=list(range(8)), **kw)
    out = np.empty((4, DIM, N), dtype=np.float32)
    for c in range(8):
        b, half = divmod(c, 2)
        out[b][:, half * NQ : (half + 1) * NQ] = res.results[c]["out"]
    return out, res


def kernel(**inputs):
    out, _ = run(
        inputs["x"], inputs["w_qkv"], inputs["w_out"], inputs["b_out"]
    )
    return out



# revision 51
# speedup vs baseline: 1.0299x; 1.0299x over previous
"""Distributed (8-core) Trainium2 Bass kernel for nn_Attention.

Reference computation (per batch b of 4, x: [4, 256, 2048]):
  qkv = w_qkv @ x[b]            -> q,k,v each [8 heads, 64, 2048]
  dots = (q^T k) * 64**-0.5     -> [8, 2048, 2048]
  attn = softmax(dots, -1)
  av   = v @ attn^T             -> [8, 64, 2048]
  out  = w_out @ av + b_out     -> [256, 2048]

Sharding: 8 shards = (batch b in 0..3) x (query-half in 0..1). Each core
gets the full x[b] (columns permuted so its own 1024 query positions come
first), computes full k/v (duplicated with its half-partner), q only for
its 1024 queries, its half of the attention, and its half of the final
projection. Host concatenates.

Per-core softmax exp is the bottleneck (16.8M elements; ScalarE alone
would take ~147us). It is split between ScalarE (exact ACT exp,
~1147ns per [128,1024] tile) and DVE (3 of every 16 tiles; Schraudolph
bit-trick exp: one tensor_scalar computing int16(round(d*A + B)) whose
bits ARE bf16(exp(d*SCALE)) with a +-3% sawtooth error that averages
out across keys; ~1.2us/tile). The sawtooth mean is zeroed via the B
constant so mixing exact and approx tiles inside one softmax row does
not bias the attention weights. DVE-exp tiles' AV matmuls are emitted
one jc late so the in-order PE FIFO never waits on the DVE queue.

Softmax normalization: both heads' denominator rows are copied to a
partition-0 [1,1024] tile (the reciprocal custom uop breaks at base
partition != 0 on HW), one batched reciprocal_approx_fast, then the
reciprocal row is broadcast across 64 partitions by bouncing through
DRAM with a 0-stride-source DMA (gpsimd's partition_broadcast is
loadable ext-isa ucode that thrashes a ~6us IRAM reload against its
resident tensor_tensor; a 0-stride SBUF DMA source is illegal), and
the per-head multiply runs on gpsimd (resident ucode, exact). The last
group instead broadcasts via a PE fp32 outer product and multiplies on
DVE to keep the tail chain short.

Emission is paced: the v-projection, next pair's q/k projections,
normalization chain and output projection are drained one-per-jc inside
the attention jc loops so the PE/DVE/ScalarE pipelines never see a burst
at group boundaries. PSUM: 4 banks dots (double-buffered [128,1024]),
2 banks AV accumulators, 2 banks projections (so a projection matmul
never WAR-serializes on the previous piece's cast). Junk warm-up
matmuls run during the input DMA window so the PE HAM clock-gate is
released before the first real matmul burst.
"""

import sys

sys.path.insert(0, "/opt/trn_rl_repo")
sys.path.insert(0, "/root/.axon_site")

from collections import deque

import numpy as np

DIM = 256
N = 2048
NQ = 1024
H = 8
DH = 64
HID = 512
PAIRS = 4
SCALE = DH ** -0.5

# Schraudolph exp constants: int16 bits of bf16(exp(d*SCALE)) ~=
# round(d * A_S + B_S).  B centered so the relative-error sawtooth is
# mean-zero (c = -0.0564 -> 128*c = -7.2193).
A_S = 128.0 * np.log2(np.e) * SCALE
B_S = 127.0 * 128.0 - 7.2192745

# jc indices (of 16) whose exp runs on DVE instead of ScalarE
DVE_JC = (3, 8, 13)

_CACHE = {}


def _register_ntff_hook():
    """The agent image's antenv lacks axon_hooks; synthesize it so
    run_bass_kernel_spmd(trace=True) can profile. Harmless if unused."""
    import types

    if "antenv.axon_hooks" in sys.modules:
        return
    try:
        import antenv
        from trn_agent_boot.trn_boot import _ntff_profile_via_ctypes

        mod = types.ModuleType("antenv.axon_hooks")
        _hook = [None]
        mod.set_axon_ntff_profile_hook = lambda h: _hook.__setitem__(0, h)
        mod.get_axon_ntff_profile_hook = lambda: _hook[0]
        sys.modules["antenv.axon_hooks"] = mod
        antenv.axon_hooks = mod
        mod.set_axon_ntff_profile_hook(
            _ntff_profile_via_ctypes("/opt/axon/libaxon_pjrt.so")
        )
    except Exception:
        pass


def build_nc():
    import concourse.mybir as mybir
    import concourse.tile as tile
    from concourse import bacc

    f32 = mybir.dt.float32
    bf16 = mybir.dt.bfloat16
    i16 = mybir.dt.int16
    Exp = mybir.ActivationFunctionType.Exp
    MULT = mybir.AluOpType.mult
    ADD = mybir.AluOpType.add

    nc = bacc.Bacc("TRN2", target_bir_lowering=False, debug=False)

    x_ext = nc.dram_tensor("x", [DIM, N], bf16, kind="ExternalInput")
    # wq_t | wk_t | wv_t packed along columns: fewer, larger startup DMAs
    wqkv_ext = nc.dram_tensor("wqkv_t", [DIM, 3 * HID], bf16, kind="ExternalInput")
    wo_ext = nc.dram_tensor("wo_t", [HID, DIM], bf16, kind="ExternalInput")
    b_ext = nc.dram_tensor("bias", [DIM, 1], f32, kind="ExternalInput")
    out_ext = nc.dram_tensor("out", [DIM, NQ], bf16, kind="ExternalOutput")
    # DRAM bounce buffer for the softmax-denominator reciprocal broadcast:
    # row g holds [recA | recB] for group g
    rb_ext = nc.dram_tensor("rbounce", [8, 1024], f32, kind="Internal")

    # v slot layout: [ones | 63 dead cols | 64 v cols]. The AV matmul then
    # writes [128, 512] with the softmax denominator on PSUM partition 0
    # (the reciprocal custom uop only works at base partition 0, so it can
    # read the den row directly -- no staging copies) and the values at
    # partitions 64:128 (a legal quadrant base for the normalize multiply).
    # Dead cols are never read downstream; m-width is free on the PE
    # (cycles = rhs columns).
    VSLOT = 2 * DH

    with tile.TileContext(nc) as tc:
        with (
            tc.tile_pool(name="persist", bufs=1) as pp,
            tc.tile_pool(name="qk", bufs=2) as qk,
            tc.tile_pool(name="epool", bufs=5) as ep,
            tc.tile_pool(name="eipool", bufs=3) as eip,
            tc.tile_pool(name="small", bufs=4) as sp,
            tc.tile_pool(name="pdots", bufs=2, space="PSUM") as pd,
            tc.tile_pool(name="pattn", bufs=2, space="PSUM") as pa,
            # bufs=2: a projection matmul must not serialize behind the
            # previous piece's PSUM->SBUF cast (WAR on a single bank)
            tc.tile_pool(name="pproj", bufs=2, space="PSUM") as pj,
        ):
            # ---- input DMAs (bf16, host pre-transposed/cast) ----
            # NOTE: emitted before the ACT-table warm-up — the scalar ring
            # carries x, and its descriptors must not queue behind the
            # ~2.7us ACT_TABLE_LOAD
            def load_bf16(ext, rows, cols, tag):
                tiles = []
                h = cols // 2
                for cc in range(rows // 128):
                    t = pp.tile([128, cols], bf16, tag=f"{tag}{cc}", name=f"{tag}{cc}")
                    r0, r1 = cc * 128, (cc + 1) * 128
                    nc.sync.dma_start(t[:, 0:h], ext[r0:r1, 0:h])
                    nc.gpsimd.dma_start(t[:, h:cols], ext[r0:r1, h:cols])
                    tiles.append(t)
                return tiles

            # Priority load order. Group (0,0) starts once wqkv q0/k0 cols
            # + x[*, 0:512] land, and consumes all v-columns + x
            # progressively (jit v + k pieces), so: scalar ring carries
            # x[*, 0:512] first (256KB, the critical path), sync/gpsimd
            # carry each cc's pair-0-critical wqkv columns
            # [q0 0:128 | k0 512:640 | v 1024:1536], then the remaining x
            # halves, then the rest of wqkv, wo and bias.
            wqkv_sb = [
                pp.tile([128, 3 * HID], bf16, tag=f"wqkv{c}", name=f"wqkv{c}")
                for c in range(2)
            ]
            x_sb = [pp.tile([128, N], bf16, tag=f"x{c}", name=f"x{c}") for c in range(2)]
            # Priority load order. Group (0,0) starts once wqkv q0/k0 cols
            # + x[*, 0:512] land, and consumes all v-columns + x
            # progressively (jit v + k pieces), so: scalar ring carries
            # x[*, 0:512] first (256KB, the critical path), sync/gpsimd
            # carry each cc's pair-0-critical wqkv columns
            # [q0 0:128 | k0 512:640 | v 1024:1536], then the remaining x
            # halves, then the rest of wqkv, wo and bias.
            engs = [nc.sync, nc.gpsimd]
            for c in range(2):
                r0, r1 = c * 128, (c + 1) * 128
                nc.scalar.dma_start(x_sb[c][:, 0:512], x_ext[r0:r1, 0:512])
                engs[c].dma_start(
                    wqkv_sb[c][:, 0:128], wqkv_ext[r0:r1, 0:128]
                )
                engs[c].dma_start(
                    wqkv_sb[c][:, HID : HID + 128],
                    wqkv_ext[r0:r1, HID : HID + 128],
                )
                engs[c].dma_start(
                    wqkv_sb[c][:, 2 * HID : 3 * HID],
                    wqkv_ext[r0:r1, 2 * HID : 3 * HID],
                )
                engs[c].dma_start(
                    x_sb[c][:, 512:1024], x_ext[r0:r1, 512:1024]
                )
            nc.scalar.dma_start(x_sb[0][:, 1024:2048], x_ext[0:128, 1024:2048])
            nc.gpsimd.dma_start(x_sb[1][:, 1024:1536], x_ext[128:256, 1024:1536])
            nc.scalar.dma_start(x_sb[1][:, 1536:2048], x_ext[128:256, 1536:2048])
            for c in range(2):
                r0, r1 = c * 128, (c + 1) * 128
                engs[c].dma_start(
                    wqkv_sb[c][:, 128:HID], wqkv_ext[r0:r1, 128:HID]
                )
                engs[c].dma_start(
                    wqkv_sb[c][:, HID + 128 : 2 * HID],
                    wqkv_ext[r0:r1, HID + 128 : 2 * HID],
                )
            wo_sb = load_bf16(wo_ext, HID, DIM, "wo")
            bias_sb = pp.tile([128, 2], f32, tag="bias")
            for oc in range(2):
                nc.sync.dma_start(
                    bias_sb[:, oc : oc + 1], b_ext[oc * 128 : (oc + 1) * 128, :]
                )
            # ---- warm the ACT exp table early (one tiny op) ----
            dummy = sp.tile([1, 1], f32, tag="dummy")
            nc.vector.memset(dummy[:], 0.0)
            dummy2 = sp.tile([1, 1], f32, tag="dummy2")
            nc.scalar.activation(dummy2[:], dummy[:], Exp)

            # two junk matmuls fill the PE's pre-DMA dead window (~7.4-8.7us)
            # and start the HAM warm-up; the real q/k/v projections (ready
            # ~8.8us) continue it -- 8 junk matmuls measured 3.5us of
            # head-of-line delay on the first real projection
            wu = pp.tile([1, 512], bf16, tag="wu")
            nc.vector.memset(wu[:], 1.0)
            for _ in range(2):
                wps = pj.tile([64, 512], f32, tag="proj", name="wps")
                nc.tensor.matmul(
                    wps[:], lhsT=wu[:, 0:64], rhs=wu[:], start=True, stop=True
                )

            # ---- persistent tiles ----
            vt = pp.tile([128, 16 * H * VSLOT], bf16, tag="vt")
            # ones column FIRST in each slot so the AV accumulator's
            # denominator row lands on PSUM partition 0 (reciprocal custom
            # uop requires base partition 0; lets it read PSUM directly)
            vt_slots = vt[:].rearrange("p (s v) -> p s v", v=VSLOT)
            nc.vector.memset(vt_slots[:, :, 0:1], 1.0)

            attn_n = [
                pp.tile([128, NQ], bf16, tag=f"attn_n{p}", name=f"attn_n{p}")
                for p in range(PAIRS)
            ]
            ones64b = pp.tile([1, DH], bf16, tag="ones64b")
            nc.vector.memset(ones64b[:], 1.0)
            out_acc = [
                pp.tile([128, NQ], f32, tag=f"oacc{oc}", name=f"oacc{oc}")
                for oc in range(2)
            ]

            # ---- q/k projection pieces (matmul and cast as separate paced
            # steps: the cast runs a jc later so it never makes the DVE
            # queue wait on a just-issued matmul) ----
            def qk_mm(p, q_t, k_t, which, pool, tag, box):
                kind, col = which
                off = 0 if kind == "q" else HID
                ps = pool.tile([128, 512], f32, tag=tag, name="ps")
                for cc in range(2):
                    nc.tensor.matmul(
                        ps[:],
                        lhsT=wqkv_sb[cc][:, off + p * 128 : off + (p + 1) * 128],
                        rhs=x_sb[cc][:, col : col + 512],
                        start=(cc == 0),
                        stop=(cc == 1),
                    )
                box[which] = ps

            def qk_cast(q_t, k_t, which, box):
                kind, col = which
                dst = q_t if kind == "q" else k_t
                nc.vector.tensor_copy(dst[:, col : col + 512], box.pop(which)[:])

            def qk_piece(p, q_t, k_t, which, pool, tag):
                box = {}
                qk_mm(p, q_t, k_t, which, pool, tag, box)
                qk_cast(q_t, k_t, which, box)

            def qk_steps(p, q_t, k_t, whichs, pool, tag):
                """One paced closure per projection chunk. (A split
                mm/cast interleave was tried and measured worse: the extra
                paced items pushed the DVE exp tiles further back in its
                FIFO.)"""
                return [
                    (lambda which=which: qk_piece(p, q_t, k_t, which, pool, tag))
                    for which in whichs
                ]

            def v_piece(jc):
                ps = pj.tile([128, HID], f32, tag="proj", name="ps")
                for cc in range(2):
                    nc.tensor.matmul(
                        ps[:],
                        lhsT=x_sb[cc][:, jc * 128 : (jc + 1) * 128],
                        rhs=wqkv_sb[cc][:, 2 * HID : 3 * HID],
                        start=(cc == 0),
                        stop=(cc == 1),
                    )
                vslice = vt[
                    :, jc * H * VSLOT : (jc + 1) * H * VSLOT
                ].rearrange("p (h s) -> p h s", s=VSLOT)
                nc.vector.tensor_copy(
                    vslice[:, :, DH : 2 * DH],
                    ps[:].rearrange("p (h d) -> p h d", d=DH),
                )

            # ---- pair 0 q/k: ic0-critical pieces upfront (pd pool), rest
            # paced into the first attention group (pj pool) ----
            q0_t = qk.tile([128, NQ], bf16, tag="q", name="q_t")
            k0_t = qk.tile([128, N], bf16, tag="k", name="k_t")
            qk_piece(0, q0_t, k0_t, ("q", 0), pd, "dots")
            qk_piece(0, q0_t, k0_t, ("k", 0), pd, "dots")

            def norm_chain(p, ic, att_s_pair, att_pair):
                """Normalization closures for one (p, ic). The AV ones row
                sits at PSUM/att_s partition 0, so the reciprocal custom
                uop (base-partition-0 only) runs straight on the den row:
                no den staging copies. rec is broadcast across 64
                partitions by bouncing through DRAM with a 0-stride-source
                DMA (gpsimd's partition_broadcast is ext-isa ucode that
                would thrash its resident tensor_tensor; a 0-stride SBUF
                DMA source is illegal), and the per-head multiply runs on
                gpsimd (resident ucode, exact). The last group instead
                reads everything straight from PSUM (recip + DVE muls) and
                broadcasts via a PE outer product to keep the tail short."""
                last = p == PAIRS - 1 and ic == 1
                g = 2 * p + ic
                box = {}

                den_src = att_pair if last else att_s_pair

                def make_rec(hh):
                    def s():
                        if hh == 0:
                            rec = sp.tile([1, 1024], f32, tag="rec", name="rec")
                            box["rec"] = rec
                        rec = box["rec"]
                        nc.vector.reciprocal_approx_fast(
                            rec[:, hh * 512 : (hh + 1) * 512],
                            den_src[hh][0:1, :],
                        )
                        if hh == 1 and not last:
                            nc.sync.dma_start(rb_ext[g : g + 1, :], rec[:])
                    return s

                def s_cast():
                    # last group only: bf16 rec so the PE broadcast runs
                    # at bf16 column rate (one [64,1024] matmul for both
                    # heads vs 2x 1.06us fp32)
                    rec_b = sp.tile([1, 1024], bf16, tag="rec_b")
                    nc.vector.tensor_copy(rec_b[:], box["rec"][:])
                    box["rec_b"] = rec_b

                def s_bcast_pe():
                    # two matmuls: a single one may not write across a
                    # PSUM bank boundary (1024 fp32 cols = 2 banks)
                    recb = pd.tile([128, 1024], f32, tag="dots", name="recb_ps")
                    for hh in range(2):
                        nc.tensor.matmul(
                            recb[0:64, hh * 512 : (hh + 1) * 512],
                            lhsT=ones64b[:],
                            rhs=box["rec_b"][:, hh * 512 : (hh + 1) * 512],
                            start=True,
                            stop=True,
                        )
                    box["recb"] = recb

                def make_bcast(hh):
                    def s():
                        # written at partitions 64:128: TensorTensor
                        # requires equal base partitions when both
                        # inputs are SBUF, and the att values sit at
                        # 64:128
                        recb = sp.tile([128, 512], f32, tag="recb")
                        nc.sync.dma_start(
                            recb[64:128, :],
                            rb_ext[
                                g : g + 1, hh * 512 : (hh + 1) * 512
                            ].broadcast_to([64, 512]),
                        )
                        box[f"recb{hh}"] = recb
                    return s

                def make_mul(hh):
                    def s():
                        if last:
                            # DVE with the PE-broadcast recb still in PSUM
                            # (one PSUM operand is legal, two are not)
                            nc.vector.tensor_tensor(
                                attn_n[p][
                                    hh * 64 : (hh + 1) * 64,
                                    ic * 512 : (ic + 1) * 512,
                                ],
                                att_s_pair[hh][DH : 2 * DH, :],
                                box["recb"][0:64, hh * 512 : (hh + 1) * 512],
                                mybir.AluOpType.mult,
                            )
                        else:
                            recb = box[f"recb{hh}"]
                            nc.gpsimd.tensor_tensor(
                                attn_n[p][
                                    hh * 64 : (hh + 1) * 64,
                                    ic * 512 : (ic + 1) * 512,
                                ],
                                att_s_pair[hh][DH : 2 * DH, :],
                                recb[64:128, :],
                                mybir.AluOpType.mult,
                            )
                    return s

                if last:
                    return [
                        make_rec(0), make_rec(1), s_cast, s_bcast_pe,
                        make_mul(0), make_mul(1),
                    ]
                return [
                    make_rec(0), make_rec(1),
                    make_bcast(0), make_mul(0),
                    make_bcast(1), make_mul(1),
                ]

            def outproj_steps(p, ic2):
                """Output projection for pair p, query-half ic2. The ic
                attention groups produce attn_n column halves, so the ic2=0
                projection can pace a full group earlier than ic2=1."""
                steps = []
                for oc in range(2):
                    def s(oc=oc):
                        ps = pj.tile([128, 512], f32, tag="proj", name="ops")
                        nc.tensor.matmul(
                            ps[:],
                            lhsT=wo_sb[p][:, oc * 128 : (oc + 1) * 128],
                            rhs=attn_n[p][:, ic2 * 512 : (ic2 + 1) * 512],
                            start=True,
                            stop=True,
                        )
                        dst = out_acc[oc][:, ic2 * 512 : (ic2 + 1) * 512]
                        if p == 0:
                            nc.vector.tensor_scalar_add(
                                dst, ps[:], bias_sb[:, oc : oc + 1]
                            )
                        elif p < PAIRS - 1:
                            nc.vector.tensor_add(dst, dst, ps[:])
                        else:
                            # last pair: final sum goes to a bf16 staging
                            # tile (halves the output DMA bytes) and ships
                            # split across rings. Scalar's ring only joins
                            # for the drain-time half (it is exp-busy
                            # while ic2=0 paces through group (3,1)).
                            stage = sp.tile(
                                [128, 512], bf16, tag="ostage", name="ostage"
                            )
                            nc.vector.tensor_add(stage[:], dst, ps[:])
                            orow = out_ext[oc * 128 : (oc + 1) * 128, :]
                            c0 = ic2 * 512
                            if ic2 == 0:
                                nc.sync.dma_start(
                                    orow[:, c0 : c0 + 256], stage[:, 0:256]
                                )
                                nc.gpsimd.dma_start(
                                    orow[:, c0 + 256 : c0 + 512],
                                    stage[:, 256:512],
                                )
                            else:
                                nc.sync.dma_start(
                                    orow[:, c0 : c0 + 192], stage[:, 0:192]
                                )
                                nc.gpsimd.dma_start(
                                    orow[:, c0 + 192 : c0 + 384],
                                    stage[:, 192:384],
                                )
                                nc.scalar.dma_start(
                                    orow[:, c0 + 384 : c0 + 512],
                                    stage[:, 384:512],
                                )
                    steps.append(s)
                return steps

            # ---- attention groups ----
            pending = deque()
            # remaining pair-0 projection pieces, paced into group (0, 0).
            # k first: group (0,0)'s own dots consume k chunk jc*128 at
            # iteration jc, so each k cast must drain before its first
            # reader (k512 cast lands at drain slot 3 < jc4, k1024 at 6 <
            # jc8, k1536 at 8 < jc12).
            pending.extend(
                qk_steps(
                    0, q0_t, k0_t,
                    (("k", 512), ("q", 512), ("k", 1024), ("k", 1536)),
                    pj, "proj",
                )
            )

            cur_qk = (q0_t, k0_t)
            next_qk = [None]

            def attention_group(p, ic, jit_v, pre_d=None, next_prologue=None):
                """Software-pipelined: dots(jc+1) is emitted BEFORE AV(jc)
                and every exp is emitted before any same-jc AV. Tile's
                cross-engine waits use a conservative PE-completion counter
                (everything emitted so far), so an exp emitted after an AV
                transitively waits for that AV (+ ~0.6us semaphore latency)
                -- that chain, not ScalarE throughput, set the old 1.19us
                steady period. With dots hoisted ahead, exp(jc+1)'s
                threshold covers only work that finishes during exp(jc).
                next_prologue (the next group's dots(0)) is emitted at
                jc=15 so the boundary AV(15) doesn't head-of-line block
                the next group's start."""
                # group (0,0) already loads DVE with the 16 v-copies; keep
                # its exps on ScalarE there
                dve_jc = () if jit_v else DVE_JC
                q_t, k_t = cur_qk
                attA = pa.tile([128, 512], f32, tag="att", name="attA")
                attB = pa.tile([128, 512], f32, tag="att", name="attB")

                def av_pair(jc, eA, eB):
                    base = jc * H * VSLOT
                    hA = 2 * p
                    hB = 2 * p + 1
                    nc.tensor.matmul(
                        attA[:],
                        lhsT=vt[:, base + hA * VSLOT : base + (hA + 1) * VSLOT],
                        rhs=eA,
                        start=(jc == 0),
                        stop=(jc == 15),
                    )
                    nc.tensor.matmul(
                        attB[:],
                        lhsT=vt[:, base + hB * VSLOT : base + (hB + 1) * VSLOT],
                        rhs=eB,
                        start=(jc == 0),
                        stop=(jc == 15),
                    )

                d_tiles = {}

                def emit_dots(jc):
                    if jit_v:
                        v_piece(jc)
                    d = pd.tile([128, 1024], f32, tag="dots", name="d")
                    nc.tensor.matmul(
                        d[:, 0:512],
                        lhsT=k_t[0:64, jc * 128 : (jc + 1) * 128],
                        rhs=q_t[0:64, ic * 512 : (ic + 1) * 512],
                        start=True,
                        stop=True,
                    )
                    nc.tensor.matmul(
                        d[:, 512:1024],
                        lhsT=k_t[64:128, jc * 128 : (jc + 1) * 128],
                        rhs=q_t[64:128, ic * 512 : (ic + 1) * 512],
                        start=True,
                        stop=True,
                    )
                    d_tiles[jc] = d

                if pre_d is not None:
                    d_tiles[0] = pre_d.pop("d")
                else:
                    emit_dots(0)
                deferred_av = []
                for jc in range(16):
                    d = d_tiles.pop(jc)
                    # exp first: its PE-counter threshold must not cover
                    # the AVs emitted below
                    if jc in dve_jc:
                        ei = eip.tile([128, 1024], i16, tag="ei")
                        nc.vector.tensor_scalar(
                            ei[:], d[:], A_S, B_S, MULT, ADD
                        )
                        eA = ei[:, 0:512].bitcast(bf16)
                        eB = ei[:, 512:1024].bitcast(bf16)
                        this_av = (
                            lambda jc=jc, eA=eA, eB=eB: av_pair(jc, eA, eB)
                        )
                    else:
                        e = ep.tile([128, 1024], bf16, tag="e")
                        nc.scalar.activation(e[:], d[:], Exp, scale=SCALE)
                        this_av = (
                            lambda jc=jc, e=e: av_pair(
                                jc, e[:, 0:512], e[:, 512:1024]
                            )
                        )
                    # next dots ahead of any AV in the PE FIFO
                    if jc < 15:
                        emit_dots(jc + 1)
                    elif next_prologue is not None:
                        next_prologue()
                    # a deferred AV slots in here: its exp (on DVE) got
                    # nearly two jcs of latency without stalling PE's FIFO
                    for av in deferred_av:
                        av()
                    deferred_av.clear()
                    if jc in dve_jc and jc < 15:
                        deferred_av.append(this_av)
                    else:
                        this_av()
                    # pace deferred side-steps: one per jc, plus extras if
                    # the backlog wouldn't drain by the end of the group
                    if jc >= 1:
                        popped = 0
                        while pending and (
                            popped < 1 or len(pending) > 15 - jc
                        ):
                            pending.popleft()()
                            popped += 1
                for av in deferred_av:
                    av()
                # free the accumulator banks quickly with one copy per head;
                # on DVE — moving these to ScalarE measured 3-5us worse (the
                # copy head-of-line blocks ScalarE's FIFO at group
                # boundaries while it waits for the AV stop).
                # last group: copies on ScalarE (idle by then) so the DVE
                # FIFO goes straight to the recip chain
                last = p == PAIRS - 1 and ic == 1
                att_s_pair = []
                for att in (attA, attB):
                    att_s = sp.tile([128, 512], f32, tag="att_s", name="att_s")
                    if last:
                        nc.scalar.copy(att_s[:], att[:])
                    else:
                        nc.vector.tensor_copy(att_s[:], att[:])
                    att_s_pair.append(att_s)
                return att_s_pair, (attA, attB)

            def make_prologue(p_n, ic_n, qk_n):
                """Next group's dots(0), emitted at the current group's
                jc=15 so it runs in the last exp's shadow instead of
                queueing behind the boundary AV(15)."""
                box = {}

                def prologue():
                    q_t, k_t = qk_n
                    d = pd.tile([128, 1024], f32, tag="dots", name="d")
                    nc.tensor.matmul(
                        d[:, 0:512],
                        lhsT=k_t[0:64, 0:128],
                        rhs=q_t[0:64, ic_n * 512 : (ic_n + 1) * 512],
                        start=True,
                        stop=True,
                    )
                    nc.tensor.matmul(
                        d[:, 512:1024],
                        lhsT=k_t[64:128, 0:128],
                        rhs=q_t[64:128, ic_n * 512 : (ic_n + 1) * 512],
                        start=True,
                        stop=True,
                    )
                    box["d"] = d

                return prologue, box

            groups = [(p, ic) for p in range(PAIRS) for ic in range(2)]
            pre_d = None
            for gi, (p, ic) in enumerate(groups):
                if ic == 0 and p < PAIRS - 1:
                    # queue next pair's q/k projection pieces: first 3
                    # paced into this (p, ic0) group, rest into (p, ic1).
                    # Group (0,0) is already loaded with the jit
                    # v-projection, so pair 1's pieces all go to (0,1).
                    nq_t = qk.tile([128, NQ], bf16, tag="q", name="q_t")
                    nk_t = qk.tile([128, N], bf16, tag="k", name="k_t")
                    next_qk[0] = (nq_t, nk_t)
                    first = () if p == 0 else (("q", 0), ("k", 0), ("q", 512))
                    if first:
                        pending.extend(
                            qk_steps(p + 1, nq_t, nk_t, first, pj, "proj")
                        )
                if ic == 1 and p < PAIRS - 1:
                    nq_t, nk_t = next_qk[0]
                    rest = (
                        (("q", 0), ("k", 0), ("q", 512),
                         ("k", 512), ("k", 1024), ("k", 1536))
                        if p == 0
                        else (("k", 512), ("k", 1024), ("k", 1536))
                    )
                    pending.extend(
                        qk_steps(p + 1, nq_t, nk_t, rest, pj, "proj")
                    )
                if gi + 1 < len(groups):
                    p_n, ic_n = groups[gi + 1]
                    qk_n = cur_qk if ic_n == 1 else next_qk[0]
                    next_prologue, next_box = make_prologue(p_n, ic_n, qk_n)
                else:
                    next_prologue, next_box = None, None
                att_s_pair, att_pair = attention_group(
                    p, ic, jit_v=(p == 0 and ic == 0),
                    pre_d=pre_d, next_prologue=next_prologue,
                )
                pre_d = next_box
                for s in norm_chain(p, ic, att_s_pair, att_pair):
                    pending.append(s)
                if p == PAIRS - 1 and ic == 0:
                    # last pair: the first query-half's output projection
                    # paces into group (3,1) so the final drain only holds
                    # norm(3,1) + outproj(3,1)
                    for s in outproj_steps(p, 0):
                        pending.append(s)
                if ic == 1:
                    # both query halves' projections pace into the next
                    # pair's first group (pacing them earlier measured
                    # worse: they crowd the qk-piece groups)
                    if p == PAIRS - 1:
                        for s in outproj_steps(p, 1):
                            pending.append(s)
                    else:
                        for s in outproj_steps(p, 0) + outproj_steps(p, 1):
                            pending.append(s)
                    if p < PAIRS - 1:
                        cur_qk = next_qk[0]
            # drain whatever is left (last pair's norm + output projection)
            while pending:
                pending.popleft()()

    nc.compile()
    return nc


def _shard_inputs(x, w_qkv, w_out, b_out):
    """Returns in_maps for cores 0..7; core c = (batch c//2, query-half c%2)."""
    x = np.asarray(x, dtype=np.float32)
    w_qkv = np.asarray(w_qkv, dtype=np.float32)
    w_out = np.asarray(w_out, dtype=np.float32)
    b_out = np.asarray(b_out, dtype=np.float32)

    import ml_dtypes

    bf = ml_dtypes.bfloat16
    # wq_t | wk_t | wv_t packed: [256, 1536]
    wqkv_t = np.ascontiguousarray(w_qkv.T).astype(bf)
    wo_t = np.ascontiguousarray(w_out.T).astype(bf)  # [512, 256]
    bias = np.ascontiguousarray(b_out.reshape(DIM, 1))

    in_maps = []
    for c in range(8):
        b, half = divmod(c, 2)
        xb = x[b]
        halves = [xb[:, 0:NQ], xb[:, NQ:N]]
        x_perm = np.ascontiguousarray(
            np.concatenate([halves[half], halves[1 - half]], axis=1)
        ).astype(ml_dtypes.bfloat16)
        in_maps.append(
            {
                "x": x_perm,
                "wqkv_t": wqkv_t,
                "wo_t": wo_t,
                "bias": bias,
            }
        )
    return in_maps


def run(x, w_qkv, w_out, b_out, trace=False, tmpdir=None):
    from concourse.bass_utils import run_bass_kernel_spmd

    _register_ntff_hook()
    if "nc" not in _CACHE:
        _CACHE["nc"] = build_nc()
    nc = _CACHE["nc"]
    in_maps = _shard_inputs(x, w_qkv, w_out, b_out)
    kw = {}
    if trace:
        kw.update(trace=True, tmpdir=tmpdir)
    res = run_bass_kernel_spmd(nc, in_maps, core_ids=list(range(8)), **kw)
    out = np.empty((4, DIM, N), dtype=np.float32)
    for c in range(8):
        b, half = divmod(c, 2)
        out[b][:, half * NQ : (half + 1) * NQ] = res.results[c]["out"]
    return out, res


def kernel(**inputs):
    out, _ = run(
        inputs["x"], inputs["w_qkv"], inputs["w_out"], inputs["b_out"]
    )
    return out

